# revision 1
# baseline (speedup 1.0000x reference)
"""Trainium2 Bass kernel: 8-head MultiHeadAttention (B=4, N=2048, E=512).

Sharding: 8 cores = 4 batches x 2 query-halves (data parallel). Each core
computes K/V for its whole batch (keys ordered own-half-first, other-half
second -- softmax is invariant to key permutation as long as K and V agree),
attention for its 1024 queries x all 8 heads, and its slice of the output
projection. No collectives; the host concatenates the 8 [1024, 512] slices.

Device-side design:
  - All matmul operands are float32r (full PE rate; plain float32 matmuls run
    at 1/4 rate; measured end-to-end relative error ~2e-4).
  - Projections produce feature-major tensors (K^T/Q^T: [head*64+d, tok]) so
    attention scores are computed directly as S^T = K' @ Q^T with keys on
    PSUM partitions (the 1/sqrt(64) scale is folded into wk/bk on the host).
  - Head pairs are processed together: both heads' scores for a q-half go
    into one [128,1024] PSUM tile at PE row groups (0,0)/(64,0), so the two
    K=64 matmuls run concurrently in the array, and a single exp (scalar
    engine, straight out of PSUM, free dim 1024) covers both heads.
  - V is produced token-major with a fused ones-column (V' = [V_h | 1]) so
    the attention-output matmul also yields the softmax denominators free.
  - Normalization multiplies ctx rows by reciprocal denominators broadcast
    across partitions with a K=1 matmul (ones[1,64]^T @ recip[1,q]).
  - Scheduling: the in-order PE stream is kept fed by emitting the next
    pair's K/Q projections, the previous pair's normalization, and the
    previous pair's partial output projection as "fillers" inside the
    current pair's kc loop; DMA loads are ordered to match first use.
    PSUM budget (8 banks) = 2x [128,1024] score tiles + 4x [128,512] slots
    shared by AV accumulators / projection groups / broadcasts / finals.
"""

import os
import sys

import numpy as np

for _p in ("/opt/trn_rl_repo", "/root/.axon_site/_ro/trn_rl_repo"):
    if os.path.isdir(_p) and _p not in sys.path:
        sys.path.insert(0, _p)

import concourse.bass as bass
from concourse import bacc
import concourse.tile as tile
from concourse import mybir
from concourse.bass_utils import run_bass_kernel_spmd

P = 128          # partitions
E = 512          # embed dim
H = 8            # heads
DH = 64          # head dim
T = 2048         # tokens per batch
NQ = 1024        # queries per core
FC = 4           # contraction chunks (512 / 128)
EC = 4           # output-feature chunks
KC = 16          # key-token chunks (2048 / 128)
B = 4
N_CORES = 8

F32 = mybir.dt.float32
F32R = mybir.dt.float32r
ADD = mybir.AluOpType.add
MUL = mybir.AluOpType.mult
EXP = mybir.ActivationFunctionType.Exp


def build_nc(passes=1, packed=True, dbl_scores=False):
    nc = bacc.Bacc(trn_type="TRN2")

    xq = nc.declare_dram_parameter("xq", [E, NQ], F32R, isOutput=False)
    xo = nc.declare_dram_parameter("xo", [E, NQ], F32R, isOutput=False)
    wqt = nc.declare_dram_parameter("wqt", [E, E], F32R, isOutput=False)
    wkt = nc.declare_dram_parameter("wkt", [E, E], F32R, isOutput=False)
    wvt = nc.declare_dram_parameter("wvt", [E, E], F32R, isOutput=False)
    wot = nc.declare_dram_parameter("wot", [E, E], F32R, isOutput=False)
    bqp = nc.declare_dram_parameter("bqp", [P, EC], F32, isOutput=False)
    bkp = nc.declare_dram_parameter("bkp", [P, EC], F32, isOutput=False)
    bvb = nc.declare_dram_parameter("bvb", [P, E], F32, isOutput=False)
    bob = nc.declare_dram_parameter("bob", [P, E], F32, isOutput=False)
    out = nc.declare_dram_parameter("out", [NQ, E], F32, isOutput=True)

    with tile.TileContext(nc) as tc:
        with (
            tc.tile_pool(name="const", bufs=1) as cp,
            tc.tile_pool(name="attn", bufs=1) as atp,
            tc.tile_pool(name="kq", bufs=2) as kqp,
            tc.tile_pool(name="vpool", bufs=1) as vpp,
            tc.tile_pool(name="pin", bufs=1) as pin,
            tc.tile_pool(name="exps", bufs=3) as xsp,
            tc.tile_pool(name="exph", bufs=5) as xhp,
            tc.tile_pool(name="norm", bufs=2) as nrm,
            tc.tile_pool(name="osb", bufs=1) as osb,
            tc.tile_pool(name="psA", bufs=2, space="PSUM") as psA,
            tc.tile_pool(name="psO", bufs=4, space="PSUM") as psO,
        ):
            for _pass in range(passes):
                # ---------- input loads ----------
                # DMA order tracks the PE consumption order (QT0, KT0, vp, ...)
                # so the in-order PE stream never waits long on a load.
                wk_t, xq_t, xo_t, wq_t, wv_t, wo_t = [], [], [], [], [], []
                for f in range(FC):
                    t_ = pin.tile([P, NQ], F32R, name=f"xq{f}", tag=f"xq{f}")
                    nc.sync.dma_start(t_[:, 0:E], xq[f * P:(f + 1) * P, 0:E])
                    xq_t.append(t_)
                for f in range(FC):
                    w = pin.tile([P, E], F32R, name=f"wq{f}", tag=f"wq{f}")
                    nc.sync.dma_start(w, wqt[f * P:(f + 1) * P, :])
                    wq_t.append(w)
                bq_t = cp.tile([P, EC], F32, name="bq", tag="bq")
                nc.sync.dma_start(bq_t, bqp[:, :])
                bk_t = cp.tile([P, EC], F32, name="bk", tag="bk")
                nc.sync.dma_start(bk_t, bkp[:, :])
                for f in range(FC):
                    w = pin.tile([P, E], F32R, name=f"wk{f}", tag=f"wk{f}")
                    nc.sync.dma_start(w, wkt[f * P:(f + 1) * P, :])
                    wk_t.append(w)
                for f in range(FC):
                    nc.sync.dma_start(xq_t[f][:, E:NQ], xq[f * P:(f + 1) * P, E:NQ])
                for f in range(FC):
                    w = pin.tile([P, E], F32R, name=f"wv{f}", tag=f"wv{f}")
                    nc.sync.dma_start(w, wvt[f * P:(f + 1) * P, :])
                    wv_t.append(w)
                bvb_t = cp.tile([P, E], F32, name="bvb", tag="bvb")
                nc.sync.dma_start(bvb_t, bvb[:, :])
                for f in range(FC):
                    t_ = pin.tile([P, NQ], F32R, name=f"xo{f}", tag=f"xo{f}")
                    nc.sync.dma_start(t_[:, 0:E], xo[f * P:(f + 1) * P, 0:E])
                    xo_t.append(t_)
                for f in range(FC):
                    nc.sync.dma_start(xo_t[f][:, E:NQ], xo[f * P:(f + 1) * P, E:NQ])
                for f in range(FC):
                    w = cp.tile([P, E], F32R, name=f"wo{f}", tag=f"wo{f}")
                    nc.sync.dma_start(w, wot[f * P:(f + 1) * P, :])
                    wo_t.append(w)
                bob_t = cp.tile([P, E], F32, name="bob", tag="bob")
                nc.sync.dma_start(bob_t, bob[:, :])
                ones_f = cp.tile([P, DH], F32, name="onesf", tag="onesf")
                nc.vector.memset(ones_f, 1.0)
                ones_t = cp.tile([33, DH], F32R, name="ones", tag="ones")
                nc.vector.tensor_copy(out=ones_t, in_=ones_f[0:33, :])

                # ---------- persistent activation tiles ----------
                vp = [vpp.tile([P, H, DH + 1], F32R, name=f"vp{t}", tag=f"vp{t}")
                      for t in range(KC)]
                ctx = [atp.tile([P, NQ], F32R, name=f"ctx{j}", tag=f"ctx{j}")
                       for j in range(EC)]

                def xcat(f, c0, w):
                    # token columns [c0, c0+w) of concat(xq, xo), feature chunk f
                    if c0 + w <= NQ:
                        return xq_t[f][:, c0:c0 + w]
                    return xo_t[f][:, c0 - NQ:c0 - NQ + w]

                def kt_group(kt_j, j, tcp):
                    ps = psO.tile([P, E], F32, name=f"pk{j}_{tcp}", tag="psO")
                    for f in range(FC):
                        nc.tensor.matmul(
                            ps,
                            (wk_t[f][:, j * P:(j + 1) * P]),
                            (xcat(f, tcp * E, E)),
                            start=(f == 0), stop=(f == FC - 1),
                        )
                    nc.vector.tensor_scalar_add(
                        kt_j[:, tcp * E:(tcp + 1) * E], ps, bk_t[:, j:j + 1])

                def qt_group(qt_j, j, tcp):
                    ps = psO.tile([P, E], F32, name=f"pq{j}_{tcp}", tag="psO")
                    for f in range(FC):
                        nc.tensor.matmul(
                            ps,
                            (wq_t[f][:, j * P:(j + 1) * P]),
                            (xq_t[f][:, tcp * E:(tcp + 1) * E]),
                            start=(f == 0), stop=(f == FC - 1),
                        )
                    nc.vector.tensor_scalar_add(
                        qt_j[:, tcp * E:(tcp + 1) * E], ps, bq_t[:, j:j + 1])

                def emit_vp(t):
                    ps = psO.tile([P, E], F32, name=f"pv{t}", tag="psO")
                    for f in range(FC):
                        nc.tensor.matmul(
                            ps,
                            (xcat(f, t * P, P)),
                            (wv_t[f]),
                            start=(f == 0), stop=(f == FC - 1),
                        )
                    nc.vector.tensor_tensor(
                        vp[t][:, :, 0:DH],
                        ps.rearrange("p (h d) -> p h d", d=DH),
                        bvb_t.rearrange("p (h d) -> p h d", d=DH),
                        ADD,
                    )
                    nc.vector.tensor_copy(
                        out=vp[t][:, :, DH:DH + 1], in_=ones_f[:, 0:H, None])

                def emit_head(h, kt_j, qt_j, sh, outs=None, kcs=range(KC),
                              lazy_vp=False, fillers=()):
                    j, par = h // 2, h % 2
                    fillers = list(fillers)
                    if outs is None:
                        o0 = psO.tile([DH + 1, E], F32, name=f"o0_{h}", tag="psO")
                        o1 = psO.tile([DH + 1, E], F32, name=f"o1_{h}", tag="psO")
                        outs = (o0, o1)
                    r0, r1 = par * DH, (par + 1) * DH
                    for k in kcs:
                        if lazy_vp:
                            emit_vp(k)
                        s = psA.tile([P, NQ], F32, name=f"s{h}_{k}", tag="psA")
                        for qc in range(2):
                            nc.tensor.matmul(
                                s[:, qc * E:(qc + 1) * E],
                                (kt_j[r0:r1, k * P:(k + 1) * P]),
                                (qt_j[r0:r1, qc * E:(qc + 1) * E]),
                                start=True, stop=True,
                                tile_position=(par * DH, 0),
                            )
                        ex = xsp.tile([P, NQ], F32R, name=f"ex{h}_{k}", tag="ex")
                        nc.scalar.activation(ex, s, EXP)
                        for qc in range(2):
                            nc.tensor.matmul(
                                outs[qc],
                                (vp[k][:, h, :]),
                                (ex[:, qc * E:(qc + 1) * E]),
                                start=(k == 0), stop=(k == KC - 1),
                            )
                        if fillers:
                            fillers.pop(0)()
                    while fillers:
                        fillers.pop(0)()
                    if kcs[-1] != KC - 1:
                        return outs
                    for qc, o in enumerate(outs):
                        # softmax denominators (ones-column row) -> row 32*par
                        nc.vector.tensor_copy(
                            out=sh[32 * par:32 * par + 1, qc * E:(qc + 1) * E],
                            in_=o[DH:DH + 1, :])
                        nc.vector.tensor_copy(
                            out=ctx[j][r0:r1, qc * E:(qc + 1) * E], in_=o[0:DH, :])
                    return outs

                def emit_pair(j, kt_j, qt_j, sh, fillers=(), stage0=False):
                    # Heads 2j/2j+1 together: per q-half pass, both heads'
                    # scores go into one [128,1024] PSUM tile at row groups
                    # (0,0)/(64,0) -- the PE runs them concurrently -- and
                    # one exp covers both. AV accumulates per head per pass.
                    fillers = list(fillers)
                    for qc in range(2):
                        oe = psO.tile([DH + 1, E], F32,
                                      name=f"oe{j}_{qc}", tag="psO")
                        oo = psO.tile([DH + 1, E], F32,
                                      name=f"oo{j}_{qc}", tag="psO")
                        exh = {}
                        pre = 5 if (stage0 and qc == 0) else 0
                        for k in range(pre):
                            s = psA.tile([P, NQ], F32,
                                         name=f"sp{j}_{qc}_{k}", tag="psA")
                            for par in range(2):
                                nc.tensor.matmul(
                                    s[:, par * E:(par + 1) * E],
                                    (kt_j[par * DH:(par + 1) * DH,
                                          k * P:(k + 1) * P]),
                                    (qt_j[par * DH:(par + 1) * DH,
                                          qc * E:(qc + 1) * E]),
                                    start=True, stop=True,
                                    tile_position=(par * DH, 0),
                                )
                            ex = xhp.tile([P, NQ], F32R,
                                          name=f"exp{j}_{qc}_{k}", tag="exh")
                            nc.scalar.activation(ex, s, EXP)
                            exh[k] = ex
                        if stage0 and qc == 0:
                            kt_group(kt_j, j, 2)
                            kt_group(kt_j, j, 3)
                        for k in range(pre):
                            if stage0 and qc == 0:
                                emit_vp(k)
                            for par, o in ((0, oe), (1, oo)):
                                nc.tensor.matmul(
                                    o,
                                    (vp[k][:, 2 * j + par, :]),
                                    (exh[k][:, par * E:(par + 1) * E]),
                                    start=(k == 0), stop=False,
                                )
                        for k in range(pre, KC):
                            if stage0 and qc == 0:
                                emit_vp(k)
                            s = psA.tile([P, NQ], F32,
                                         name=f"sp{j}_{qc}_{k}", tag="psA")
                            for _rep in range(2 if dbl_scores else 1):
                                for par in range(2):
                                    nc.tensor.matmul(
                                        s[:, par * E:(par + 1) * E],
                                        (kt_j[par * DH:(par + 1) * DH,
                                              k * P:(k + 1) * P]),
                                        (qt_j[par * DH:(par + 1) * DH,
                                              qc * E:(qc + 1) * E]),
                                        start=True, stop=True,
                                        tile_position=(par * DH, 0),
                                    )
                            ex = xsp.tile([P, NQ], F32R,
                                          name=f"ex{j}_{qc}_{k}", tag="ex")
                            nc.scalar.activation(ex, s, EXP)
                            for par, o in ((0, oe), (1, oo)):
                                nc.tensor.matmul(
                                    o,
                                    (vp[k][:, 2 * j + par, :]),
                                    (ex[:, par * E:(par + 1) * E]),
                                    start=(k == 0), stop=(k == KC - 1),
                                )
                            if fillers:
                                fillers.pop(0)()
                        while fillers:
                            fillers.pop(0)()
                        for par, o in ((0, oe), (1, oo)):
                            nc.vector.tensor_copy(
                                out=sh[32 * par:32 * par + 1,
                                       qc * E:(qc + 1) * E],
                                in_=o[DH:DH + 1, :])
                            nc.vector.tensor_copy(
                                out=ctx[j][par * DH:(par + 1) * DH,
                                           qc * E:(qc + 1) * E],
                                in_=o[0:DH, :])

                def normalize_fillers(j, sh):
                    # reciprocal of the pair's softmax denominators (rows
                    # 0/32), broadcast across the 64 head-dim partitions via
                    # a K=1 matmul, then scale ctx rows in place. Split into
                    # small fillers so it interleaves with the next pair.
                    rp = nrm.tile([33, NQ], F32R, name=f"rp{j}", tag="rp",
                                  bufs=1)

                    def recip():
                        with nc.allow_low_precision(
                                reason="f32r softmax denominators"):
                            nc.vector.reciprocal(rp, sh)

                    def bcast_mul(par, qc):
                        rb = psO.tile([P, E], F32,
                                      name=f"rb{2 * j + par}_{qc}", tag="psO")
                        nc.tensor.matmul(
                            rb[0:DH, :],
                            (ones_t[32 * par:32 * par + 1, :]),
                            (rp[32 * par:32 * par + 1, qc * E:(qc + 1) * E]),
                            start=True, stop=True,
                        )
                        rows = ctx[j][par * DH:(par + 1) * DH,
                                      qc * E:(qc + 1) * E]
                        nc.vector.tensor_tensor(rows, rows, rb[0:DH, :], MUL)

                    return [recip] + [
                        lambda par=par, qc=qc: bcast_mul(par, qc)
                        for qc in range(2) for par in range(2)]

                def final_fillers(j, store=False):
                    # partial output projection for head-pair j, accumulated
                    # into the 8 SBUF output tiles (bias folded into pair 0).
                    def fpass(qt_i):
                        pf = psO.tile([P, E], F32, name=f"pf{j}_{qt_i}",
                                      tag="psO")
                        nc.tensor.matmul(
                            pf,
                            (ctx[j][:, qt_i * P:(qt_i + 1) * P]),
                            (wo_t[j]),
                            start=True, stop=True,
                        )
                        if j == 0:
                            ot = osb.tile([P, E], F32, name=f"ot{qt_i}",
                                          tag=f"ot{qt_i}")
                            ot_t[qt_i] = ot
                            nc.vector.tensor_tensor(ot, pf, bob_t, ADD)
                        else:
                            ot = ot_t[qt_i]
                            nc.vector.tensor_tensor(ot, ot, pf, ADD)
                        if store:
                            nc.sync.dma_start(
                                out[qt_i * P:(qt_i + 1) * P, :], ot_t[qt_i])

                    return [lambda qt_i=qt_i: fpass(qt_i)
                            for qt_i in range(NQ // P)]

                # ---------- schedule ----------
                # pair 0 is staged against DMA arrival (scores+exp for kc 0-3
                # come before any V'/AV work so the scalar engine starts early);
                # projections for pair j+1, normalize(j-1) and the partial
                # output projection for pair j-1 run as fillers inside pair j's
                # kc loops so neither PE nor ACT stalls at pair boundaries.
                kt_n = [None] * EC
                qt_n = [None] * EC
                sh_n = [None] * EC
                ot_t = [None] * (NQ // P)

                def make_pair_fillers(jn):
                    kt_n[jn] = kqp.tile([P, T], F32R, name=f"kt{jn}", tag="kt")
                    qt_n[jn] = kqp.tile([P, NQ], F32R, name=f"qt{jn}", tag="qt")
                    fs = [lambda tcp=tcp: qt_group(qt_n[jn], jn, tcp)
                          for tcp in range(2)]
                    fs += [lambda tcp=tcp: kt_group(kt_n[jn], jn, tcp)
                           for tcp in range(4)]
                    return fs

                if packed:
                    for j in range(EC):
                        sh_n[j] = nrm.tile([33, NQ], F32, name=f"sh{j}",
                                           tag="sh")
                        nc.vector.memset(sh_n[j], 1.0)
                        fillers = []
                        if j == 0:
                            kt_n[0] = kqp.tile([P, T], F32R, name="kt0",
                                               tag="kt")
                            qt_n[0] = kqp.tile([P, NQ], F32R, name="qt0",
                                               tag="qt")
                            qt_group(qt_n[0], 0, 0)
                            kt_group(kt_n[0], 0, 0)
                            kt_group(kt_n[0], 0, 1)
                            qt_group(qt_n[0], 0, 1)
                        else:
                            fillers += normalize_fillers(j - 1, sh_n[j - 1])
                            fillers += final_fillers(j - 1)
                        if j + 1 < EC:
                            fillers += make_pair_fillers(j + 1)
                        emit_pair(j, kt_n[j], qt_n[j], sh_n[j],
                                  fillers=fillers, stage0=(j == 0))
                    nrm3 = normalize_fillers(EC - 1, sh_n[EC - 1])
                    fin3 = final_fillers(EC - 1, store=True)
                    order = [nrm3[0], nrm3[1], nrm3[2], fin3[0], fin3[1],
                             fin3[2], fin3[3], nrm3[3], nrm3[4], fin3[4],
                             fin3[5], fin3[6], fin3[7]]
                    for f in order:
                        f()
                for j in range(EC if not packed else 0):
                    sh_n[j] = nrm.tile([33, NQ], F32, name=f"sh{j}", tag="sh")
                    nc.vector.memset(sh_n[j], 1.0)
                    if j == 0:
                        kt_n[0] = kqp.tile([P, T], F32R, name="kt0", tag="kt")
                        qt_n[0] = kqp.tile([P, NQ], F32R, name="qt0", tag="qt")
                        # group order tracks DMA arrival: qt tcp0 (xq-h0 + wq),
                        # kt tcp0/1 (wk), qt tcp1 (xq-h1)
                        qt_group(qt_n[0], 0, 0)
                        kt_group(kt_n[0], 0, 0)
                        kt_group(kt_n[0], 0, 1)
                        qt_group(qt_n[0], 0, 1)
                        # head-0 prologue: per-half scores+exp for kc 0-2, no AV
                        # yet -- gets the scalar engine going as early as DMA
                        # allows (exp of q-half 0 only needs xq-h0/wq/wk).
                        o0 = psO.tile([DH + 1, E], F32, name="o0_0", tag="psO")
                        o1 = psO.tile([DH + 1, E], F32, name="o1_0", tag="psO")
                        h0_outs = (o0, o1)
                        exh = {}
                        for qc in range(2):
                            for k in range(3):
                                s = psA.tile([P, E], F32,
                                             name=f"s0_{k}_{qc}", tag="psA")
                                nc.tensor.matmul(
                                    s,
                                    (kt_n[0][0:DH, k * P:(k + 1) * P]),
                                    (qt_n[0][0:DH, qc * E:(qc + 1) * E]),
                                    start=True, stop=True, tile_position=(0, 0),
                                )
                                ex = xhp.tile([P, E], F32R,
                                              name=f"exh{k}_{qc}", tag="exh")
                                nc.scalar.activation(ex, s, EXP)
                                exh[(k, qc)] = ex
                        kt_group(kt_n[0], 0, 2)
                        kt_group(kt_n[0], 0, 3)
                        for k in range(3):
                            emit_vp(k)
                            for qc in range(2):
                                nc.tensor.matmul(
                                    h0_outs[qc],
                                    (vp[k][:, 0, :]),
                                    (exh[(k, qc)]),
                                    start=(k == 0), stop=False,
                                )
                        emit_head(0, kt_n[0], qt_n[0], sh_n[0], outs=h0_outs,
                                  kcs=range(3, KC), lazy_vp=True)
                    else:
                        fs = normalize_fillers(j - 1, sh_n[j - 1])
                        fs += final_fillers(j - 1)
                        emit_head(2 * j, kt_n[j], qt_n[j], sh_n[j], fillers=fs)
                    nxt_fillers = make_pair_fillers(j + 1) if j + 1 < EC else ()
                    emit_head(2 * j + 1, kt_n[j], qt_n[j], sh_n[j],
                              fillers=nxt_fillers)

                if not packed:
                    # tail: last pair's normalize interleaved with the final
                    # projection groups (q-tiles 0-3 only need the qc=0
                    # halves of ctx[3] normalized); stores fold into the
                    # final passes.
                    nrm3 = normalize_fillers(EC - 1, sh_n[EC - 1])
                    fin3 = final_fillers(EC - 1, store=True)
                    order = [nrm3[0], nrm3[1], nrm3[2], fin3[0], fin3[1],
                             fin3[2], fin3[3], nrm3[3], nrm3[4], fin3[4],
                             fin3[5], fin3[6], fin3[7]]
                    for f in order:
                        f()

    nc.compile()
    return nc


_NC = None


def _get_nc():
    global _NC
    if _NC is None:
        _NC = build_nc()
    return _NC


def make_in_maps(q, wq, bq, wk, bk, wv, bv, wo, bo):
    q = np.asarray(q, np.float32)
    scale = 1.0 / np.sqrt(np.float32(DH))
    shared = dict(
        wqt=np.ascontiguousarray(np.asarray(wq, np.float32).T),
        wkt=np.ascontiguousarray(np.asarray(wk, np.float32).T * scale),
        wvt=np.ascontiguousarray(np.asarray(wv, np.float32).T),
        wot=np.ascontiguousarray(np.asarray(wo, np.float32).T),
        bqp=np.ascontiguousarray(np.asarray(bq, np.float32).reshape(EC, P).T),
        bkp=np.ascontiguousarray(
            (np.asarray(bk, np.float32) * scale).reshape(EC, P).T),
        bvb=np.ascontiguousarray(
            np.broadcast_to(np.asarray(bv, np.float32), (P, E))),
        bob=np.ascontiguousarray(
            np.broadcast_to(np.asarray(bo, np.float32), (P, E))),
    )
    in_maps = []
    for c in range(N_CORES):
        b, half = c // 2, c % 2
        xT = q[b].T
        in_maps.append(dict(
            xq=np.ascontiguousarray(xT[:, half * NQ:(half + 1) * NQ]),
            xo=np.ascontiguousarray(xT[:, (1 - half) * NQ:(2 - half) * NQ]),
            **shared,
        ))
    return in_maps


def assemble(results):
    full = np.empty((B, T, E), np.float32)
    for c in range(N_CORES):
        b, half = c // 2, c % 2
        full[b, half * NQ:(half + 1) * NQ, :] = results[c]["out"]
    return full


def kernel(q, wq, bq, wk, bk, wv, bv, wo, bo):
    in_maps = make_in_maps(q, wq, bq, wk, bk, wv, bv, wo, bo)
    nc = _get_nc()
    res = run_bass_kernel_spmd(nc, in_maps, list(range(N_CORES)))
    return assemble(res.results)



# revision 10
# speedup vs baseline: 1.2199x; 1.2199x over previous
"""Trainium2 Bass kernel: 8-head MultiHeadAttention (B=4, N=2048, E=512).

Sharding: 8 cores = 4 batches x 2 head-groups (tensor parallel over heads).
Each core computes Q/K/V projections for ITS 4 heads only (w_q/w_k/w_v
column-parallel), attention for all 2048 queries x its 4 heads, and the
row-parallel slice of the output projection (contracting its 256 ctx
features).  The host sums the two partial outputs per batch (the
"all-reduce" of row-parallel w_out, done at gather time); the bias bo is
folded into the head-group-0 core's partials via its bob input (zeros on
head-group-1 cores).  This halves the projection FLOPs per core vs
query-split sharding (no duplicated K/V work).

Device-side design (per core; H=4 heads = 2 pairs):
  - All matmul operands are float32r (full PE rate).
  - Projections produce feature-major K^T/Q^T ([head*64+d, tok]) so scores
    are computed as S^T = K' @ Q^T with keys on PSUM partitions; the
    1/sqrt(64) scale is folded into wk/bk on the host.
  - Head pairs: both heads' scores for a 512-query chunk go into one
    [128,1024] PSUM tile at PE row groups (0,0)/(64,0) (concurrent in the
    array), one exp covers both heads.
  - V' = [V_h | 1] (token-major, fused ones column) so the AV matmul also
    yields softmax denominators for free (PSUM row 64).
  - exp runs mostly on the scalar engine (ACT); a slice of the key-chunks
    per query-group runs on the vector engine via a 2-op fast-exp: a
    tensor_scalar mult+add with f32->int32 convert (Schraudolph seed), then
    a custom DVE op applying a quadratic mantissa correction
    (max rel err ~0.36%, irrelevant vs the 2e-2 gate; the softmax ratio
    cancels the common mode).  This offloads the ACT bottleneck.
  - Normalization: denominators' reciprocal via the fast custom-DVE
    reciprocal, broadcast across the 64 head-dim partitions with a pair of
    K=1 column-tiled matmuls (both heads in one PSUM tile), then one
    in-place [128,512] multiply per query chunk.
  - kt/qt PSUM evacuation + bias runs on ACT (Identity with per-partition
    bias) in the prologue window where ACT would otherwise idle; exp and
    Identity share one activation table set (exp_and_others), so no table
    switches.
  - Scheduling: in-order PE stream kept fed by emitting the next pair's
    projections, the previous pair's normalization and partial output
    projection as fillers inside the current pair's key-chunk loop.
"""

import os
import sys

import numpy as np

for _p in ("/opt/trn_rl_repo", "/root/.axon_site/_ro/trn_rl_repo"):
    if os.path.isdir(_p) and _p not in sys.path:
        sys.path.insert(0, _p)

import concourse.bass as bass
from concourse import bacc
import concourse.tile as tile
from concourse import mybir
from concourse.bass_utils import run_bass_kernel_spmd

P = 128          # partitions
EIN = 512        # input feature dim
EOUT = 512       # output embed dim
F = 256          # per-core projection features (4 heads x 64)
H = 4            # heads per core
DH = 64          # head dim
T = 2048         # tokens (= keys) per batch
NQ = 2048        # queries per core
FC = 4           # input-feature chunks (512/128)
KC = 16          # key-token chunks (2048/128)
QCN = 4          # query chunks (2048/512)
NPAIR = 2        # head pairs per core
TOKC = 16        # token chunks for the output projection
B = 4
N_CORES = 8

F32 = mybir.dt.float32
F32R = mybir.dt.float32r
I32 = mybir.dt.int32
ADD = mybir.AluOpType.add
MUL = mybir.AluOpType.mult
EXP = mybir.ActivationFunctionType.Exp
IDENT = mybir.ActivationFunctionType.Identity

# ---- fast-exp constants (Schraudolph seed + quadratic mantissa fix) ----
_LN2 = float(np.log(2.0))
EXPA = float((1 << 23) / _LN2)       # scale for z = A*x + B
EXPB = float(127 * (1 << 23)) - 713696.0   # exponent bias - log2(kappa)*2^23
# (shift pins the fast-exp global scale to exactly 1 so fast and exact
#  chunks can mix inside one softmax without bias)
EXPC_AMP = 0.243644409169            # quadratic amplitude (Src1 tile)
EXPC_SHIFT = -1.483050321385         # quadratic center (imm2)
_MASKC = float(np.int32(0x007FFFFF).view(np.float32))   # mantissa mask bits
_ORC = 1.0                           # bits 0x3F800000 double as +1.0
assert np.float32(_MASKC).view(np.int32) == 0x007FFFFF

# which key-chunks per (pair, qc) run exp on the DVE instead of ACT
DVE_EXP_KS = frozenset(
    int(v) for v in os.environ.get("DVE_EXP_KS", "7,11").split(",") if v != ""
)


def _make_exp_op():
    """Register a custom DVE op: out = in0 * (in1*(m + C2)^2 + C1) with
    m = bitcast((bits(in0) & bits(C0)) | bits(C1)).  in0 is the Schraudolph
    seed y = bitcast(int32(A*x+B)); m = 1+frac reconstructs the mantissa;
    the quadratic corrects the piecewise-linear 2^f by 2^f/(1+f)."""
    import concourse.dve_ops as dvo
    from concourse.dve_spec import (
        AluOp, Bin, C0, C1, C2, Spec, Src0, Src1, lower, sq,
    )
    from concourse.dve_uop import DveOpSpec

    name = "EXP_SEED_CORR_ANT"
    if name in dvo._SUB_OPCODE_FOR_NAME:
        return next(o for o in dvo.OPS if o.name == name)

    def _ref(in0, in1, s0, s1, imm2):
        mask = np.float32(s0).view(np.int32)
        orc = np.float32(s1).view(np.int32)
        m = ((in0.view(np.int32) & mask) | orc).view(np.float32)
        return (in0 * (in1 * (m + imm2) ** 2 + s1)).astype(np.float32)

    m = Bin(AluOp.BITWISE_OR, Bin(AluOp.BITWISE_AND, Src0, C0), C1)
    spec = Spec(body=Src0 * (Src1 * sq(m + C2) + C1), reference=_ref)

    row = max(dvo._SUB_OPCODE_FOR_NAME.values()) + 1
    assert row < 0x20
    shas = {}
    for ver in ("v3", "v4"):
        try:
            probe = DveOpSpec(
                name=name, opcode=row, uops=lower(spec, ver=ver), rd1_en=True
            )
            shas[ver] = probe.sha(ver)
        except Exception:
            pass
    if not shas:
        return None
    op = dvo.DveOp(name, spec, subdim=False, uops_sha=shas)
    dvo._SUB_OPCODE_FOR_NAME[name] = row
    dvo.OPS.append(op)
    return op


try:
    EXPC_OP = _make_exp_op()
except Exception:
    EXPC_OP = None


def build_nc(passes=1, dve_exp=None):
    if dve_exp is None:
        dve_exp = EXPC_OP is not None and \
            os.environ.get("DVE_EXP", "0") == "1"
    dve_ks = DVE_EXP_KS if dve_exp else frozenset()

    nc = bacc.Bacc(trn_type="TRN2")

    xd = nc.declare_dram_parameter("xd", [EIN, T], F32R, isOutput=False)
    wqt = nc.declare_dram_parameter("wqt", [EIN, F], F32R, isOutput=False)
    wkt = nc.declare_dram_parameter("wkt", [EIN, F], F32R, isOutput=False)
    wvt = nc.declare_dram_parameter("wvt", [EIN, F], F32R, isOutput=False)
    wot = nc.declare_dram_parameter("wot", [F, EOUT], F32R, isOutput=False)
    bqp = nc.declare_dram_parameter("bqp", [P, NPAIR], F32, isOutput=False)
    bkp = nc.declare_dram_parameter("bkp", [P, NPAIR], F32, isOutput=False)
    bvb = nc.declare_dram_parameter("bvb", [P, F], F32, isOutput=False)
    bob = nc.declare_dram_parameter("bob", [P, EOUT], F32, isOutput=False)
    out = nc.declare_dram_parameter("out", [NQ, EOUT], F32, isOutput=True)

    with tile.TileContext(nc) as tc:
        with (
            tc.tile_pool(name="const", bufs=1) as cp,
            tc.tile_pool(name="pin", bufs=1) as pin,
            tc.tile_pool(name="kq", bufs=2) as kqp,
            tc.tile_pool(name="vpool", bufs=1) as vpp,
            tc.tile_pool(name="attn", bufs=1) as atp,
            tc.tile_pool(name="exps", bufs=3) as xsp,
            tc.tile_pool(name="exph", bufs=5) as xhp,
            tc.tile_pool(name="expi", bufs=1) as xip,
            tc.tile_pool(name="norm", bufs=1) as nrm,
            tc.tile_pool(name="osb", bufs=4) as osb,
            tc.tile_pool(name="psA", bufs=2, space="PSUM") as psA,
            tc.tile_pool(name="psO", bufs=4, space="PSUM") as psO,
        ):
            for _pass in range(passes):
                # ---------- input loads (ordered by first PE use) ----------
                x_t = [pin.tile([P, T], F32R, name=f"x{f}", tag=f"x{f}")
                       for f in range(FC)]
                for f in range(FC):
                    nc.sync.dma_start(x_t[f][:, 0:512], xd[f * P:(f + 1) * P, 0:512])
                wq_t = []
                for f in range(FC):
                    w = pin.tile([P, F], F32R, name=f"wq{f}", tag=f"wq{f}")
                    nc.sync.dma_start(w, wqt[f * P:(f + 1) * P, :])
                    wq_t.append(w)
                bq_t = cp.tile([P, NPAIR], F32, name="bq", tag="bq")
                nc.sync.dma_start(bq_t, bqp[:, :])
                wk_t = []
                for f in range(FC):
                    w = pin.tile([P, F], F32R, name=f"wk{f}", tag=f"wk{f}")
                    nc.sync.dma_start(w, wkt[f * P:(f + 1) * P, :])
                    wk_t.append(w)
                bk_t = cp.tile([P, NPAIR], F32, name="bk", tag="bk")
                nc.sync.dma_start(bk_t, bkp[:, :])
                for tcp in range(1, 4):
                    for f in range(FC):
                        nc.sync.dma_start(
                            x_t[f][:, tcp * 512:(tcp + 1) * 512],
                            xd[f * P:(f + 1) * P, tcp * 512:(tcp + 1) * 512])
                wv_t = []
                for f in range(FC):
                    w = pin.tile([P, F], F32R, name=f"wv{f}", tag=f"wv{f}")
                    nc.sync.dma_start(w, wvt[f * P:(f + 1) * P, :])
                    wv_t.append(w)
                bvb_t = cp.tile([P, F], F32, name="bvb", tag="bvb")
                nc.sync.dma_start(bvb_t, bvb[:, :])
                wo_t = []
                for j in range(NPAIR):
                    w = cp.tile([P, EOUT], F32R, name=f"wo{j}", tag=f"wo{j}")
                    nc.sync.dma_start(w, wot[j * P:(j + 1) * P, :])
                    wo_t.append(w)
                bob_t = cp.tile([P, EOUT], F32, name="bob", tag="bob")
                nc.sync.dma_start(bob_t, bob[:, :])

                ones_f = cp.tile([P, DH], F32, name="onesf", tag="onesf")
                nc.vector.memset(ones_f, 1.0)
                ones_t = cp.tile([33, DH], F32R, name="ones", tag="ones")
                nc.vector.tensor_copy(out=ones_t, in_=ones_f[0:33, :])
                amp_t = cp.tile([P, 1], F32, name="amp", tag="amp")
                nc.vector.memset(amp_t, EXPC_AMP)

                # ---------- persistent activation tiles ----------
                vp = [vpp.tile([P, H, DH + 1], F32R, name=f"vp{t}", tag=f"vp{t}")
                      for t in range(KC)]
                ctx = [atp.tile([P, NQ], F32R, name=f"ctx{j}", tag=f"ctx{j}")
                       for j in range(NPAIR)]
                kt_n = [None] * NPAIR
                qt_n = [None] * NPAIR
                sh_n = [None] * NPAIR
                rp_n = [None] * NPAIR

                def kt_group(j, tcp):
                    ps = psO.tile([P, 512], F32, name=f"pk{j}_{tcp}", tag="psO")
                    for f in range(FC):
                        nc.tensor.matmul(
                            ps,
                            (wk_t[f][:, j * P:(j + 1) * P]),
                            (x_t[f][:, tcp * 512:(tcp + 1) * 512]),
                            start=(f == 0), stop=(f == FC - 1),
                        )
                    nc.scalar.activation(
                        kt_n[j][:, tcp * 512:(tcp + 1) * 512], ps, IDENT,
                        bias=bk_t[:, j:j + 1])

                def qt_group(j, tcp):
                    ps = psO.tile([P, 512], F32, name=f"pq{j}_{tcp}", tag="psO")
                    for f in range(FC):
                        nc.tensor.matmul(
                            ps,
                            (wq_t[f][:, j * P:(j + 1) * P]),
                            (x_t[f][:, tcp * 512:(tcp + 1) * 512]),
                            start=(f == 0), stop=(f == FC - 1),
                        )
                    nc.scalar.activation(
                        qt_n[j][:, tcp * 512:(tcp + 1) * 512], ps, IDENT,
                        bias=bq_t[:, j:j + 1])

                def emit_vp(t):
                    ps = psO.tile([P, F], F32, name=f"pv{t}", tag="psO")
                    for f in range(FC):
                        nc.tensor.matmul(
                            ps,
                            (x_t[f][:, t * P:(t + 1) * P]),
                            (wv_t[f]),
                            start=(f == 0), stop=(f == FC - 1),
                        )
                    nc.vector.tensor_tensor(
                        vp[t][:, :, 0:DH],
                        ps.rearrange("p (h d) -> p h d", d=DH),
                        bvb_t.rearrange("p (h d) -> p h d", d=DH),
                        ADD,
                    )
                    nc.vector.tensor_copy(
                        out=vp[t][:, :, DH:DH + 1], in_=ones_f[:, 0:H, None])

                def emit_exp(ex_, s):
                    nc.scalar.activation(ex_, s, EXP)

                def emit_exp_dve(ex_, s):
                    yi = xip.tile([P, 1024], I32, name="yi", tag="yi")
                    nc.vector.tensor_scalar(yi, s, EXPA, EXPB, MUL, ADD)
                    nc.vector._custom_dve(
                        EXPC_OP, out=ex_, in0=yi[:, :].bitcast(F32),
                        in1=amp_t[:, 0:1], s0=_MASKC, s1=_ORC, imm2=EXPC_SHIFT)

                def emit_pair(j, fillers=(), stage0=False):
                    fillers = list(fillers)
                    kt_j, qt_j, sh_j = kt_n[j], qt_n[j], sh_n[j]
                    for qc in range(QCN):
                        oe = psO.tile([DH + 1, 512], F32,
                                      name=f"oe{j}_{qc}", tag="psO")
                        oo = psO.tile([DH + 1, 512], F32,
                                      name=f"oo{j}_{qc}", tag="psO")
                        pre = 5 if (stage0 and qc == 0) else 0
                        exh = {}
                        for k in range(pre):
                            s = psA.tile([P, 1024], F32,
                                         name=f"sp{j}_{qc}_{k}", tag="psA")
                            for par in range(2):
                                nc.tensor.matmul(
                                    s[:, par * 512:(par + 1) * 512],
                                    (kt_j[par * DH:(par + 1) * DH,
                                          k * P:(k + 1) * P]),
                                    (qt_j[par * DH:(par + 1) * DH,
                                          qc * 512:(qc + 1) * 512]),
                                    start=True, stop=True,
                                    tile_position=(par * DH, 0),
                                )
                            ex = xhp.tile([P, 1024], F32R,
                                          name=f"exp{j}_{qc}_{k}", tag="exh")
                            emit_exp(ex, s)
                            exh[k] = ex
                        if stage0 and qc == 0:
                            kt_group(j, 2)
                            kt_group(j, 3)
                            qt_group(j, 1)
                        for k in range(pre):
                            emit_vp(k)
                            for par, o in ((0, oe), (1, oo)):
                                nc.tensor.matmul(
                                    o,
                                    (vp[k][:, 2 * j + par, :]),
                                    (exh[k][:, par * 512:(par + 1) * 512]),
                                    start=(k == 0), stop=False,
                                )
                        for k in range(pre, KC):
                            if stage0 and qc == 0:
                                emit_vp(k)
                            s = psA.tile([P, 1024], F32,
                                         name=f"s{j}_{qc}_{k}", tag="psA")
                            for par in range(2):
                                nc.tensor.matmul(
                                    s[:, par * 512:(par + 1) * 512],
                                    (kt_j[par * DH:(par + 1) * DH,
                                          k * P:(k + 1) * P]),
                                    (qt_j[par * DH:(par + 1) * DH,
                                          qc * 512:(qc + 1) * 512]),
                                    start=True, stop=True,
                                    tile_position=(par * DH, 0),
                                )
                            ex = xsp.tile([P, 1024], F32R,
                                          name=f"ex{j}_{qc}_{k}", tag="ex")
                            if k in dve_ks:
                                emit_exp_dve(ex, s)
                            else:
                                emit_exp(ex, s)
                            for par, o in ((0, oe), (1, oo)):
                                nc.tensor.matmul(
                                    o,
                                    (vp[k][:, 2 * j + par, :]),
                                    (ex[:, par * 512:(par + 1) * 512]),
                                    start=(k == 0), stop=(k == KC - 1),
                                )
                            if fillers:
                                fillers.pop(0)()
                        # evacuate ctx rows + denominators for this q-chunk
                        for par, o in ((0, oe), (1, oo)):
                            nc.vector.tensor_copy(
                                out=ctx[j][par * DH:(par + 1) * DH,
                                           qc * 512:(qc + 1) * 512],
                                in_=o[0:DH, :])
                            nc.vector.tensor_copy(
                                out=sh_j[32 * par:32 * par + 1,
                                         qc * 512:(qc + 1) * 512],
                                in_=o[DH:DH + 1, :])
                    while fillers:
                        fillers.pop(0)()

                def normalize_fillers(j):
                    def recip():
                        # rp is f32r (matmul moving operand); call the custom
                        # op directly -- the f32r out AP rounds on write,
                        # which the BIR verifier requires for f32r matmuls.
                        from concourse.dve_ops import (
                            RECIP_APPROX_FAST_CONSTS, RECIPROCAL_APPROX_FAST)
                        c = RECIP_APPROX_FAST_CONSTS
                        nc.vector._custom_dve(
                            RECIPROCAL_APPROX_FAST, out=rp_n[j][:, :],
                            in0=sh_n[j][:, :], s0=c["s0"], s1=c["s1"],
                            imm2=c["imm2"])

                    def bcast_mul(par, qc):
                        rb = psO.tile([P, 512], F32, name=f"rb{j}{par}_{qc}",
                                      tag="psO")
                        nc.tensor.matmul(
                            rb[0:DH, :],
                            (ones_t[32 * par:32 * par + 1, :]),
                            (rp_n[j][32 * par:32 * par + 1,
                                     qc * 512:(qc + 1) * 512]),
                            start=True, stop=True,
                        )
                        rows = ctx[j][par * DH:(par + 1) * DH,
                                      qc * 512:(qc + 1) * 512]
                        nc.vector.tensor_tensor(rows, rows, rb[0:DH, :], MUL)

                    return [recip] + [
                        lambda par=par, qc=qc: bcast_mul(par, qc)
                        for qc in range(QCN) for par in range(2)]

                def final_fillers():
                    def fpass(i):
                        pf = psO.tile([P, EOUT], F32, name=f"pf{i}",
                                      tag="psO")
                        nc.tensor.matmul(
                            pf, (ctx[0][:, i * P:(i + 1) * P]), (wo_t[0]),
                            start=True, stop=False)
                        nc.tensor.matmul(
                            pf, (ctx[1][:, i * P:(i + 1) * P]), (wo_t[1]),
                            start=False, stop=True)
                        ot = osb.tile([P, EOUT], F32, name=f"ot{i}",
                                      tag="ot")
                        nc.vector.tensor_tensor(ot, pf, bob_t, ADD)
                        nc.sync.dma_start(out[i * P:(i + 1) * P, :], ot)

                    return [lambda i=i: fpass(i) for i in range(TOKC)]

                def make_pair(jn):
                    kt_n[jn] = kqp.tile([P, T], F32R, name=f"kt{jn}", tag="kt")
                    qt_n[jn] = kqp.tile([P, NQ], F32R, name=f"qt{jn}", tag="qt")
                    sh_n[jn] = nrm.tile([33, NQ], F32, name=f"sh{jn}", tag="sh")
                    rp_n[jn] = nrm.tile([33, NQ], F32R, name=f"rp{jn}", tag="rp")
                    nc.vector.memset(sh_n[jn], 1.0)

                def make_pair_fillers(jn):
                    fs = [lambda: make_pair(jn)]
                    fs += [lambda tcp=tcp: qt_group(jn, tcp) for tcp in range(QCN)]
                    fs += [lambda tcp=tcp: kt_group(jn, tcp) for tcp in range(4)]
                    return fs

                # ---------- schedule ----------
                make_pair(0)
                qt_group(0, 0)
                kt_group(0, 0)
                kt_group(0, 1)
                fillers0 = [lambda: qt_group(0, 2), lambda: qt_group(0, 3)]
                fillers0 += make_pair_fillers(1)
                emit_pair(0, fillers=fillers0, stage0=True)

                emit_pair(1, fillers=normalize_fillers(0))

                nrm1 = normalize_fillers(1)
                fin1 = final_fillers()
                order = nrm1[0:3] + fin1[0:4] + nrm1[3:5] + fin1[4:8] + \
                    nrm1[5:7] + fin1[8:12] + nrm1[7:9] + fin1[12:16]
                for fn in order:
                    fn()

    nc.compile()
    return nc


_NC = None


def _get_nc():
    global _NC
    if _NC is None:
        _NC = build_nc()
    return _NC


def make_in_maps(q, wq, bq, wk, bk, wv, bv, wo, bo):
    q = np.asarray(q, np.float32)
    scale = np.float32(1.0 / np.sqrt(np.float32(DH)))
    wq = np.asarray(wq, np.float32)
    wk = np.asarray(wk, np.float32)
    wv = np.asarray(wv, np.float32)
    wo = np.asarray(wo, np.float32)
    bo_b = np.ascontiguousarray(
        np.broadcast_to(np.asarray(bo, np.float32), (P, EOUT)))
    zero_b = np.zeros((P, EOUT), np.float32)
    in_maps = []
    for c in range(N_CORES):
        b, hg = c // 2, c % 2
        sl = slice(hg * F, (hg + 1) * F)
        in_maps.append(dict(
            xd=np.ascontiguousarray(q[b].T),
            wqt=np.ascontiguousarray(wq[sl, :].T),
            wkt=np.ascontiguousarray(wk[sl, :].T * scale),
            wvt=np.ascontiguousarray(wv[sl, :].T),
            wot=np.ascontiguousarray(wo[:, sl].T),
            bqp=np.ascontiguousarray(
                np.asarray(bq, np.float32)[sl].reshape(NPAIR, P).T),
            bkp=np.ascontiguousarray(
                (np.asarray(bk, np.float32)[sl] * scale).reshape(NPAIR, P).T),
            bvb=np.ascontiguousarray(
                np.broadcast_to(np.asarray(bv, np.float32)[sl], (P, F))),
            bob=bo_b if hg == 0 else zero_b,
        ))
    return in_maps


def assemble(results):
    full = np.empty((B, T, EOUT), np.float32)
    for b in range(B):
        full[b] = results[2 * b]["out"]
        full[b] += results[2 * b + 1]["out"]
    return full


def kernel(q, wq, bq, wk, bk, wv, bv, wo, bo):
    in_maps = make_in_maps(q, wq, bq, wk, bk, wv, bv, wo, bo)
    nc = _get_nc()
    res = run_bass_kernel_spmd(nc, in_maps, list(range(N_CORES)))
    return assemble(res.results)
